# revision 62
# baseline (speedup 1.0000x reference)
"""Trainium2 Bass kernel for causal multi-head attention (B=2, S=2048, D=2048, H=16).

Sharding: DP over batch (2) x TP over heads (4 groups of 4 heads) = 8 cores.
Each core computes, for its batch b and head group g:
  - Q/K/V projections restricted to its 512 head-dims (transposed layouts)
  - causal attention for its 4 heads (scores produced directly transposed
    with ragged 128-granular causality on the diagonal chunk; softmax
    denominators via DVE-grouped exp sums + one ones-matmul per group; fast
    approx reciprocal; normalization folded in post-PV)
  - partial output projection through its 512 columns of wo (bf16 partials)
Host sums the 4 TP partials per batch (the "all-reduce" of the hint, done in
numpy on the gathered partials) and stacks the 2 batches.

Scheduling: proj/wo matmul groups are emission-interleaved between the
attention head phases so PE stays fed while ACT runs the exp streams; input
DMAs issue two chunks ahead on SP, out-DMAs on gpsimd, PSUM->SBUF copies on
ACT. All matmuls run in bf16 (fp32 PSUM accumulation); host pre-converts
inputs.
"""

import numpy as np
import ml_dtypes

import concourse.bacc as bacc
import concourse.tile as tile
from concourse import mybir
from concourse.bass_utils import run_bass_kernel_spmd

BF16 = ml_dtypes.bfloat16

# Full problem sizes (hardcoded; grading calls kernel() with these shapes).
B, S, D, H = 2, 2048, 2048, 16
HD = 128          # head dim
P = 128           # SBUF partitions
CH = 512          # matmul moving-dim chunk
N_CORES = 8
M_CORE = D // 4   # head-dims per core (4 heads x 128)


def build_core_kernel(nc, io, S=S, D=D, M_CORE=M_CORE):
    """Emit the per-core Tile program (single causal-chunk pipeline).

    For each 512-wide sequence chunk c: project q/k/v for chunk c, run
    attention for all heads on queries of chunk c (keys/values from chunks
    <= c, which are already projected), then the wo partial for chunk c's
    rows. This keeps TensorE dense (projection matmuls of chunk c+1 fill the
    ACT-bound stretches of attention on chunk c).
    """
    E_T = D // P        # e (contraction) tiles
    S_T = S // P        # s tiles
    S_C = S // CH       # s chunks
    H_C = M_CORE // P   # heads on this core
    D_C = D // CH       # output e chunks
    SUB = CH // P       # 128-tiles per chunk
    bf = mybir.dt.bfloat16
    f32 = mybir.dt.float32
    SCALE = 1.0 / float(np.sqrt(np.float32(HD)))

    qT, kT, vT = io["qT"], io["kT"], io["vT"]
    wqT, wkT, wvT, woT = io["wqT"], io["wkT"], io["wvT"], io["woT"]
    cmask = io["cmask"]
    out = io["out"]

    import contextlib

    with tile.TileContext(nc) as tc, contextlib.ExitStack() as ctx:
        wpool = ctx.enter_context(tc.tile_pool(name="wpool", bufs=1))
        cons = ctx.enter_context(tc.tile_pool(name="cons", bufs=1))
        projo = ctx.enter_context(tc.tile_pool(name="projo", bufs=1))
        xin = ctx.enter_context(tc.tile_pool(name="xin", bufs=34))
        xqp = ctx.enter_context(tc.tile_pool(name="xqp", bufs=2))
        expp = ctx.enter_context(tc.tile_pool(name="expt", bufs=2))
        smp = ctx.enter_context(tc.tile_pool(name="sm", bufs=2))
        outp = ctx.enter_context(tc.tile_pool(name="outp", bufs=6))
        # PSUM budget is 8 banks: 3 acc + 2 sc + 2 pv + 1 bc. acc=3 keeps the
        # proj/wo matmul groups from stalling on their PSUM->SBUF copy chain;
        # bc=1 is enough now that the reciprocal is fast (its reader drains
        # within the next head's first few tiles).
        acc = ctx.enter_context(tc.tile_pool(name="ps_acc", bufs=3, space="PSUM"))
        pssc = ctx.enter_context(tc.tile_pool(name="ps_sc", bufs=2, space="PSUM"))
        pspv = ctx.enter_context(tc.tile_pool(name="ps_pv", bufs=2, space="PSUM"))
        psbc = ctx.enter_context(tc.tile_pool(name="ps_bc", bufs=1, space="PSUM"))

        # inputs arrive pre-tiled by the host: x: [E_T, S_C, P, CH] with each
        # [P, CH] tile a contiguous 128KB block (max DMA efficiency)
        x_tiled = {"q": qT, "k": kT, "v": vT}

        # projection weights resident (per-e-tile DMAs on the idle gpsimd
        # queue so they don't contend with the input stream)
        w_sbs = {}
        for name, w_dram in (("q", wqT), ("k", wkT), ("v", wvT)):
            w_sb = wpool.tile([P, E_T, M_CORE], bf, name=f"w{name}_sb")
            wt = w_dram.rearrange("(e p) m -> e p m", p=P)
            for e_t in range(E_T):
                if name == "q" and e_t >= E_T - 2:
                    # chunk 0's first matmul sweep is gated by this train's
                    # last arrival; halving the final tiles' transfers ends
                    # the train ~2us sooner
                    h2 = M_CORE // 2
                    nc.gpsimd.dma_start(out=w_sb[:, e_t, :h2],
                                        in_=wt[e_t][:, :h2])
                    nc.gpsimd.dma_start(out=w_sb[:, e_t, h2:],
                                        in_=wt[e_t][:, h2:])
                else:
                    nc.gpsimd.dma_start(out=w_sb[:, e_t, :], in_=wt[e_t])
            w_sbs[name] = w_sb

        mask_sb = cons.tile([P, SUB, CH], bf, name="mask_sb")
        nc.gpsimd.dma_start(out=mask_sb, in_=cmask.rearrange("p (s c) -> p s c", s=SUB))
        ones_mat = cons.tile([P, P], bf, name="ones_mat")
        nc.vector.memset(ones_mat, 1.0)
        woT_sb = cons.tile([P, H_C, D], bf, name="woT_sb")
        grpp = ctx.enter_context(tc.tile_pool(name="grpp", bufs=2))

        xkT_sb = projo.tile([P, H_C, S], bf, name="xkT_sb")
        xv_sb = projo.tile([P, S_T, M_CORE], bf, name="xv_sb")
        attnT_sb = projo.tile([P, H_C, S], bf, name="attnT_sb")
        xq_ch = {}  # chunk -> per-chunk xq tile [P, H_C, CH]

        x_sb = {}  # chunk -> {name: [tiles]}

        def emit_proj_dma(c):
            """Issue chunk c's input-tile DMAs (decoupled from the matmuls so
            the SP queue runs ~2 chunks ahead of consumption)."""
            x_sb[c] = {}
            for name in ("q", "k", "v"):
                xts = []
                for e_t in range(E_T):
                    xt = xin.tile([P, CH], bf, tag="xin", name=f"x{name}{c}_{e_t}")
                    if c == 0 and name == "q" and e_t >= E_T - 2:
                        # end chunk 0's arrival-gated q train sooner (see
                        # the matching wq split above)
                        nc.sync.dma_start(out=xt[:, :CH // 2],
                                          in_=x_tiled[name][e_t, c][:, :CH // 2])
                        nc.sync.dma_start(out=xt[:, CH // 2:],
                                          in_=x_tiled[name][e_t, c][:, CH // 2:])
                    else:
                        nc.sync.dma_start(out=xt, in_=x_tiled[name][e_t, c])
                    xts.append(xt)
                x_sb[c][name] = xts

        def proj_items(c):
            """The 12 projection matmul-group closures for chunk c, in
            dependency order (q feeds sc first, v consumed last)."""
            csl = slice(c * CH, (c + 1) * CH)

            def qk_group(name, m):
                w_sb = w_sbs[name]
                xts = x_sb[c][name]
                if name == "q" and m == 0:
                    xq_ch[c] = xqp.tile([P, H_C, CH], bf, tag="xqc",
                                        name=f"xq_{c}")
                ps = acc.tile([P, CH], f32, tag="acc", name=f"ps_{name}{c}_{m}")
                for e_t in range(E_T):
                    nc.tensor.matmul(ps, lhsT=w_sb[:, e_t, m * P:(m + 1) * P],
                                     rhs=xts[e_t],
                                     start=(e_t == 0), stop=(e_t == E_T - 1))
                if name == "q":
                    nc.scalar.copy(xq_ch[c][:, m, :], ps)
                else:
                    nc.scalar.copy(xkT_sb[:, m, csl], ps)

            def v_group(s_sub):
                w_sb = w_sbs["v"]
                xts = x_sb[c]["v"]
                ps = acc.tile([P, M_CORE], f32, tag="acc", name=f"ps_v{c}_{s_sub}")
                for e_t in range(E_T):
                    nc.tensor.matmul(ps,
                                     lhsT=xts[e_t][:, s_sub * P:(s_sub + 1) * P],
                                     rhs=w_sb[:, e_t, :],
                                     start=(e_t == 0), stop=(e_t == E_T - 1))
                nc.scalar.copy(xv_sb[:, c * SUB + s_sub, :], ps)

            items = []
            for name in ("q", "k"):
                for m in range(H_C):
                    items.append(lambda name=name, m=m: qk_group(name, m))
            for s_sub in range(SUB):
                items.append(lambda s_sub=s_sub: v_group(s_sub))
            return items

        def emit_proj0():
            """Chunk 0's projection, e-outer/m-inner: its input tiles are
            still streaming in, so accumulate the first 3 output groups in
            parallel across the acc ring and consume each tile on arrival
            (instead of each m-group waiting for the full 16-tile train),
            then a 4th full-rate pass over the resident tiles."""
            for name in ("q", "k", "v"):
                w_sb = w_sbs[name]
                xts = x_sb[0][name]
                shape = M_CORE if name == "v" else CH
                if name == "q":
                    xq_ch[0] = xqp.tile([P, H_C, CH], bf, tag="xqc",
                                        name="xq_0")

                def mm(ps, m, e_t):
                    if name in ("q", "k"):
                        nc.tensor.matmul(ps, lhsT=w_sb[:, e_t, m * P:(m + 1) * P],
                                         rhs=xts[e_t],
                                         start=(e_t == 0), stop=(e_t == E_T - 1))
                    else:
                        nc.tensor.matmul(ps,
                                         lhsT=xts[e_t][:, m * P:(m + 1) * P],
                                         rhs=w_sb[:, e_t, :],
                                         start=(e_t == 0), stop=(e_t == E_T - 1))

                def store(ps, m):
                    if name == "q":
                        nc.scalar.copy(xq_ch[0][:, m, :], ps)
                    elif name == "k":
                        nc.scalar.copy(xkT_sb[:, m, 0:CH], ps)
                    else:
                        nc.scalar.copy(xv_sb[:, m, :], ps)

                ps3 = [acc.tile([P, shape], f32, tag="acc",
                                name=f"p0_{name}_{m}") for m in range(3)]
                for e_t in range(E_T):
                    for m in range(3):
                        mm(ps3[m], m, e_t)
                for m in range(3):
                    store(ps3[m], m)
                ps = acc.tile([P, shape], f32, tag="acc", name=f"p0_{name}_3")
                for e_t in range(E_T):
                    mm(ps, 3, e_t)
                store(ps, 3)

        def emit_attn(c, work):
            """Attention for chunk c; between head phases, drain a share of
            `work` (proj/wo matmul-group closures) so PE has independent
            matmuls queued while ACT grinds through the exp streams."""
            csl = slice(c * CH, (c + 1) * CH)
            n_t = SUB * (c + 1)
            n_ph = H_C + 1
            # head-offset software pipeline: head h's ACT-paced scores stream
            # is interleaved with head h-1's bc/pv matmuls (whose exp tiles
            # are already materialized), keeping TensorE ~full during the
            # exp-gated stretches. expT is double-buffered for this.
            prev = None
            done = 0
            for h in range(H_C + 1):
                share = len(work) * (h + 1) // n_ph
                while done < share:
                    work[done]()
                    done += 1
                cur = None
                if h < H_C:
                    exp_t = expp.tile([P, S_T, CH], bf, tag="expT",
                                      name=f"exp_{h}_{c}")
                    cur = (h, exp_t)
                if prev is not None:
                    ph, pexp = prev
                    ps_bc = psbc.tile([P, CH], f32, tag="bc", name=f"bc_{ph}_{c}")
                    ps_pv = pspv.tile([P, CH], f32, tag="pv", name=f"pv_{ph}_{c}")
                    n_g = n_t // 4
                    grp = None
                for t in range(n_t):
                    # ragged causality: key tile t only attends queries
                    # >= 128*t, so diagonal-chunk tiles compute a shrinking
                    # q-range [q0:512] instead of the full 512 (saves PE on
                    # sc/pv, ACT on exp). Only the 128-wide block at q0 is
                    # triangular; the rest of the range is fully visible.
                    if cur is not None:
                        q0 = max(0, t * P - c * CH)
                        ps_sc = pssc.tile([P, CH], f32, tag="sc",
                                          name=f"sc_{h}_{c}_{t}")
                        nc.tensor.matmul(ps_sc[:, q0:],
                                         lhsT=xkT_sb[:, h, t * P:(t + 1) * P],
                                         rhs=xq_ch[c][:, h, q0:],
                                         start=True, stop=True)
                        nc.scalar.activation(exp_t[:, t, q0:], ps_sc[:, q0:],
                                             mybir.ActivationFunctionType.Exp,
                                             scale=SCALE)
                        if t >= SUB * c:
                            nc.vector.tensor_mul(exp_t[:, t, q0:q0 + P],
                                                 exp_t[:, t, q0:q0 + P],
                                                 mask_sb[:, 0, 0:P])
                    if prev is not None:
                        # softmax denominator: pre-sum each group of 4 exp
                        # tiles on DVE (bf16), then one ones-matmul per group
                        # (4x fewer PE instructions than per-tile bc matmuls).
                        # In the ragged (diagonal) group, tile t contributes
                        # only [pq0:]; below pq0 its exp buffer is garbage.
                        g, r = divmod(t, 4)
                        pq0 = max(0, t * P - c * CH)
                        if r == 1:
                            grp = grpp.tile([P, CH], bf, tag="grp",
                                            name=f"grp_{ph}_{c}_{g}")
                            if pq0 > 0:
                                nc.vector.tensor_copy(grp[:, :pq0],
                                                      pexp[:, t - 1, :pq0])
                                nc.vector.tensor_add(grp[:, pq0:],
                                                     pexp[:, t - 1, pq0:],
                                                     pexp[:, t, pq0:])
                            else:
                                nc.vector.tensor_add(grp, pexp[:, t - 1, :],
                                                     pexp[:, t, :])
                        elif r > 1:
                            nc.vector.tensor_add(grp[:, pq0:], grp[:, pq0:],
                                                 pexp[:, t, pq0:])
                            if r == 3:
                                nc.tensor.matmul(ps_bc, lhsT=ones_mat, rhs=grp,
                                                 start=(g == 0),
                                                 stop=(g == n_g - 1))
                        nc.tensor.matmul(ps_pv[:, pq0:],
                                         lhsT=xv_sb[:, t, ph * P:(ph + 1) * P],
                                         rhs=pexp[:, t, pq0:],
                                         start=(t == 0), stop=(t == n_t - 1),
                                         skip_group_check=True)
                if prev is not None:
                    bc_sb = smp.tile([P, CH], f32, tag="bcs", name=f"bcs_{ph}_{c}")
                    nc.vector.reciprocal_approx_fast(out=bc_sb, in_=ps_bc)
                    nc.vector.tensor_mul(attnT_sb[:, ph, csl], ps_pv, bc_sb)
                prev = cur

        def wo_items(c):
            """The 16 wo matmul-group closures for chunk c."""
            last = c == S_C - 1
            big = None
            if last:
                # attention is over by now, so the exp ring is dead: borrow
                # one of its 16KB buffers as a flat 16-slot staging area.
                # Every final tile stays live until its DMA completes, so no
                # copy ever waits on an out-DMA (the outp-ring stall that
                # paces the final wo stretch otherwise).
                big = expp.tile([P, S_T, CH], bf, tag="expT", name="wo_tail")

            def wo_group(s_t, e_c):
                ps = acc.tile([P, CH], f32, tag="acc", name=f"wo_{s_t}_{e_c}")
                for h in range(H_C):
                    nc.tensor.matmul(ps,
                                     lhsT=attnT_sb[:, h, s_t * P:(s_t + 1) * P],
                                     rhs=woT_sb[:, h, e_c * CH:(e_c + 1) * CH],
                                     start=(h == 0), stop=(h == H_C - 1))
                if last:
                    ot = big[:, (s_t - c * SUB) * D_C + e_c, :]
                    if e_c % 2:
                        # tail: DVE is idle; alternating copies across
                        # ACT/DVE halves the copy latency in the acc ring
                        nc.vector.tensor_copy(ot, ps)
                    else:
                        nc.scalar.copy(ot, ps)
                else:
                    ot = outp.tile([P, CH], bf, tag="out",
                                   name=f"out_{s_t}_{e_c}")
                    nc.scalar.copy(ot, ps)
                osl = out[s_t * P:(s_t + 1) * P, e_c * CH:(e_c + 1) * CH]
                if last and s_t == c * SUB + SUB - 1 and e_c >= D_C - 2:
                    # final two tiles: halve the last transfers and put them
                    # on separate engines' queues so the drain after the last
                    # matmul is as short as possible
                    nc.sync.dma_start(out=osl[:, :CH // 2], in_=ot[:, :CH // 2])
                    nc.gpsimd.dma_start(out=osl[:, CH // 2:],
                                        in_=ot[:, CH // 2:])
                else:
                    # out-DMAs issue on gpsimd's DGE so they never block the
                    # SP queue's input-tile prefetch stream
                    nc.gpsimd.dma_start(out=osl, in_=ot)

            return [lambda s_t=s_t, e_c=e_c: wo_group(s_t, e_c)
                    for s_t in range(c * SUB, (c + 1) * SUB)
                    for e_c in range(D_C)]

        # software-pipelined chunk loop: proj(c+1) and wo(c-1) matmul groups
        # are interleaved between attn(c)'s head phases, so PE always has
        # independent matmuls queued while ACT grinds the exp streams (and a
        # straggling input DMA only stalls work that was deferred anyway).
        # Input DMAs are issued two chunks ahead of their matmuls.
        emit_proj_dma(0)
        emit_proj_dma(1)
        emit_proj0()
        for c in range(S_C):
            if c == 0:
                # wo weights: needed from the first wo stage on; DMA'd here so
                # they don't delay the chunk-0 input stream
                for h in range(H_C):
                    nc.gpsimd.dma_start(out=woT_sb[:, h, :],
                                        in_=woT[h * P:(h + 1) * P, :])
            if c + 2 < S_C:
                emit_proj_dma(c + 2)
            work = []
            if c > 0:
                work += wo_items(c - 1)
            if c + 1 < S_C:
                work += proj_items(c + 1)
            emit_attn(c, work)
        for it in wo_items(S_C - 1):
            it()


def build_nc(S=S, D=D, M_CORE=M_CORE):
    nc = bacc.Bacc("TRN2", target_bir_lowering=False, debug=False, num_devices=N_CORES)
    bf = mybir.dt.bfloat16
    xshape = [D // P, S // CH, P, CH]
    io = {
        "qT": nc.dram_tensor("qT", xshape, bf, kind="ExternalInput").ap(),
        "kT": nc.dram_tensor("kT", xshape, bf, kind="ExternalInput").ap(),
        "vT": nc.dram_tensor("vT", xshape, bf, kind="ExternalInput").ap(),
        "wqT": nc.dram_tensor("wqT", [D, M_CORE], bf, kind="ExternalInput").ap(),
        "wkT": nc.dram_tensor("wkT", [D, M_CORE], bf, kind="ExternalInput").ap(),
        "wvT": nc.dram_tensor("wvT", [D, M_CORE], bf, kind="ExternalInput").ap(),
        "woT": nc.dram_tensor("woT", [M_CORE, D], bf, kind="ExternalInput").ap(),
        "cmask": nc.dram_tensor("cmask", [P, (CH // P) * CH], bf,
                                kind="ExternalInput").ap(),
        "out": nc.dram_tensor("out", [S, D], mybir.dt.bfloat16,
                              kind="ExternalOutput").ap(),
    }
    build_core_kernel(nc, io, S=S, D=D, M_CORE=M_CORE)
    nc.compile()
    return nc


def make_mask():
    i = np.arange(P)[:, None]
    j = np.arange(CH)[None, :]
    m = np.concatenate(
        [(j >= P * p + i).astype(np.float32) for p in range(CH // P)], axis=1)
    return m.astype(BF16)


def tile_T(xT, D_=D, S_=S):
    """[D, S] bf16 -> tiled [D/P, S/CH, P, CH], each tile contiguous."""
    return np.ascontiguousarray(
        xT.reshape(D_ // P, P, S_ // CH, CH).transpose(0, 2, 1, 3))


def tile_xT(x):
    """[S, D] fp32 -> transposed+tiled [D/P, S/CH, P, CH] bf16."""
    return tile_T(x.T.astype(BF16))


def prep_in_maps(q, k, v, wq, wk, wv, wo):
    cmask = make_mask()
    qT = [tile_xT(q[b]) for b in range(B)]
    kT = [tile_xT(k[b]) for b in range(B)]
    vT = [tile_xT(v[b]) for b in range(B)]
    in_maps = []
    for c in range(N_CORES):
        b, g = divmod(c, N_CORES // B)
        M = slice(g * M_CORE, (g + 1) * M_CORE)
        in_maps.append({
            "qT": qT[b], "kT": kT[b], "vT": vT[b],
            "wqT": np.ascontiguousarray(wq[M, :].T).astype(BF16),
            "wkT": np.ascontiguousarray(wk[M, :].T).astype(BF16),
            "wvT": np.ascontiguousarray(wv[M, :].T).astype(BF16),
            "woT": np.ascontiguousarray(wo[:, M].T).astype(BF16),
            "cmask": cmask,
        })
    return in_maps


def run(inputs, trace=False):
    nc = build_nc()
    in_maps = prep_in_maps(inputs["q"], inputs["k"], inputs["v"],
                           inputs["wq"], inputs["wk"], inputs["wv"], inputs["wo"])
    res = run_bass_kernel_spmd(nc, in_maps, core_ids=list(range(N_CORES)),
                               trace=trace)
    g = N_CORES // B
    out = np.stack([
        np.sum([res.results[b * g + i]["out"].astype(np.float32)
                for i in range(g)], axis=0)
        for b in range(B)
    ]).astype(np.float32)
    return out, res


def kernel(**inputs):
    out, _ = run(inputs, trace=False)
    return out

